# revision 18
# baseline (speedup 1.0000x reference)
"""Fused causal MHA kernel for TRN2, one core = (batch b, head-group g of 8 heads).

v2: chunk-major wave schedule.  All four head-pairs process query-chunk c
("wave c") before moving to chunk c+1, so the W_O projection for chunk c
becomes pumpable filler during wave 3 instead of a starved endgame.  Inputs
arrive via a handful of wide folded DMAs (one descriptor each); the 1/sqrt(dk)
scale rides the exp activation; output is f16.

Layouts (per core):
  xtf  [128, 16384]  X[b]^T folded chunk-major:
        xtf[p, cc*4096 + k*512 + c] = X^T[k*128+p, cc*512+c]
  wvf  [128, 4096]   wvf[p, k*512+j]          = Wv[k*128+p, g*512+j]
  wqf/wkf [128,4096] wqf[p, hp*1024+k*128+c]  = Wq[k*128+p, g*512+hp*128+c]
  wof  [128, 4096]   wof[p, dv*1024+o]        = Wo[g*512+dv*128+p, o]
  maskt [128, 128]   mask[0:128, 0:128].T  (0 / -1e9)
  outt [1024, N] f16 partial (X attn Wo_g)^T ; host sums the two
        head-group partials per batch and transposes.

On-chip per head-pair hp: qt/kt [128, N] (h0 d0-63, h1 d0-63 on partitions).
v per seq m-block: [128, 8*65]; S^T per (hp, c, jb): psum [128, 1024];
PV col-packed into psO [65, 512] per head with a ones row for the denom.
"""

import numpy as np
import concourse.bass as bass
import concourse.tile as tile
from concourse import bacc, mybir

F32 = mybir.dt.float32
F16 = mybir.dt.float16
AF = mybir.ActivationFunctionType

P = 128
D = 1024
DH = 512  # head-group width: 8 heads * 64
DK = 64
KB = D // P  # 8 k-blocks
NHP = 4  # head-pairs per core


def build(N=2048):
    MB = N // P  # seq 128-blocks (16)
    MC = N // 512  # seq 512-chunks (4)
    nc = bacc.Bacc("TRN2", target_bir_lowering=False, debug=False)

    xtf_d = nc.dram_tensor("xtf", [P, KB * N], F16, kind="ExternalInput")
    wqf_d = nc.dram_tensor("wqf", [P, NHP * 1024], F16, kind="ExternalInput")
    wkf_d = nc.dram_tensor("wkf", [P, NHP * 1024], F16, kind="ExternalInput")
    wvf_d = nc.dram_tensor("wvf", [P, KB * DH], F16, kind="ExternalInput")
    wof_d = nc.dram_tensor("wof", [P, NHP * D], F16, kind="ExternalInput")
    mask_d = nc.dram_tensor("maskt", [P, P], F32, kind="ExternalInput")
    ones_d = nc.dram_tensor("ones16", [P, DK], F16, kind="ExternalInput")
    out_d = nc.dram_tensor("outt", [D, N], F16, kind="ExternalOutput")

    with tile.TileContext(nc) as tc:
        with (
            tc.tile_pool(name="sb", bufs=1) as sb,
            tc.tile_pool(name="ps", bufs=1, space="PSUM") as ps,
        ):
            # ---- persistent tiles ----
            xtf = sb.tile([P, KB * N], F16, tag="xtf", bufs=1)
            wqf = sb.tile([P, NHP * 1024], F16, tag="wqf", bufs=1)
            wkf = sb.tile([P, NHP * 1024], F16, tag="wkf", bufs=1)
            wvf = sb.tile([P, KB * DH], F16, tag="wvf", bufs=1)
            wof = sb.tile([P, NHP * D], F16, tag="wof", bufs=1)
            maskt = sb.tile([P, P], F32, tag="maskt", bufs=1)
            ones = sb.tile([P, DK], F16, tag="ones", bufs=1)
            v = [sb.tile([P, 8 * 65], F16, tag="v", bufs=MB, name=f"v{m}") for m in range(MB)]
            qt = [sb.tile([P, N], F16, tag="qt", bufs=NHP, name=f"qt{h}") for h in range(NHP)]
            kt = [sb.tile([P, N], F16, tag="kt", bufs=NHP, name=f"kt{h}") for h in range(NHP)]
            ot = [sb.tile([P, N], F16, tag="ot", bufs=NHP, name=f"ot{t}") for t in range(NHP)]

            # ---- input DMAs: earliest-needed first, interleaved with prologue
            # emission so the first v_proj matmul only queues behind its own
            # inputs.  xtf is folded m-block-major:
            # xtf[p, m*1024 + k*128 + c] = X^T[k*128+p, m*128+c].
            for k in range(KB):
                nc.sync.dma_start(
                    wvf[:, k * 512:(k + 1) * 512], wvf_d.ap()[:, k * 512:(k + 1) * 512]
                )

            def dma_xtf_m(m, eng=None):
                (eng or nc.sync).dma_start(
                    xtf[:, m * 1024:(m + 1) * 1024],
                    xtf_d.ap()[:, m * 1024:(m + 1) * 1024],
                )

            dma_xtf_m(0)
            nc.gpsimd.dma_start(maskt[:], mask_d.ap())
            nc.gpsimd.dma_start(ones[:], ones_d.ap())

            # warm the ACT exp table during the DMA lead-in
            warm = sb.tile([P, DK], F16, tag="warm", bufs=1, name="warm")
            nc.scalar.activation(warm[:], ones[:], AF.Exp)

            xtf3 = xtf[:].rearrange("p (m y) -> p m y", y=1024)

            def xts_chunk(k, cc):
                """xt[k][:, cc*512:(cc+1)*512] as a [128, 4, 128] strided AP."""
                return xtf3[:, 4 * cc:4 * cc + 4, k * P:(k + 1) * P]

            def v_proj_parts(m):
                """values projection for seq block m -> v[m]; 2 thunks."""
                cell = {}

                def part(k0, k1, fin):
                    if k0 == 0:
                        cell["ps"] = ps.tile([P, 512], F32, tag="proj", bufs=2, name="psV")
                    psV = cell["ps"]
                    for k in range(k0, k1):
                        nc.tensor.matmul(
                            psV[:],
                            xtf3[:, m, k * P:(k + 1) * P],
                            wvf[:, k * 512:(k + 1) * 512],
                            start=(k == 0),
                            stop=(k == KB - 1),
                        )
                    if fin:
                        v3 = v[m][:].rearrange("p (h x) -> p h x", x=65)
                        nc.vector.tensor_copy(
                            v3[:, :, 0:64], psV[:].rearrange("p (h x) -> p h x", x=64)
                        )
                        nc.gpsimd.tensor_copy(v3[:, :, 64:65], ones[:, 0:8, None])

                return [lambda: part(0, 4, False), lambda: part(4, KB, True)]

            def qk_proj_parts(hp, c, wf, dst):
                """one 512-chunk of the Q or K projection for head-pair hp; 2 thunks."""
                cell = {}

                def part(k0, k1, fin):
                    if k0 == 0:
                        cell["ps"] = ps.tile([P, 512], F32, tag="proj", bufs=2, name="psQ")
                    psQ = cell["ps"]
                    for k in range(k0, k1):
                        nc.tensor.matmul(
                            psQ[:],
                            wf[:, hp * 1024 + k * P:hp * 1024 + (k + 1) * P],
                            xts_chunk(k, c),
                            start=(k == 0),
                            stop=(k == KB - 1),
                        )
                    if fin:
                        nc.vector.tensor_copy(dst[:, c * 512:(c + 1) * 512], psQ[:])

                return [lambda: part(0, 4, False), lambda: part(4, KB, True)]

            def outproj_unit(do, c):
                """one 128-row out-dim block of chunk c of the W_O projection."""

                def run():
                    psF = ps.tile([P, 512], F32, tag="proj", bufs=2, name="psF")
                    for dv in range(NHP):
                        nc.tensor.matmul(
                            psF[:],
                            wof[:, dv * D + do * P:dv * D + (do + 1) * P],
                            ot[dv][:, c * 512:(c + 1) * 512],
                            start=(dv == 0),
                            stop=(dv == NHP - 1),
                        )
                    o_sb = sb.tile([P, 512], F16, tag="osb", bufs=4, name="o_sb")
                    nc.vector.tensor_copy(o_sb[:], psF[:])
                    nc.sync.dma_start(
                        out_d.ap()[do * P:(do + 1) * P, c * 512:(c + 1) * 512],
                        o_sb[:],
                    )

                return run

            # ---- pump state --------------------------------------------------
            filler = []  # tensor-work thunks allotted to the current wave
            dve_q = []  # norm-chain stages, popped one per attention unit
            wave_state = {"done": 0, "total": 1, "quota": 0, "popped": 0}

            def pump():
                if dve_q:
                    dve_q.pop(0)()
                wave_state["done"] += 1
                target = wave_state["quota"] * wave_state["done"] // wave_state["total"]
                while wave_state["popped"] < target and filler:
                    filler.pop(0)[1]()
                    wave_state["popped"] += 1

            # ---- attention chunk (hp, c) ------------------------------------
            def attn_chunk(hp, c, eager_norm=False):
                jb_max = min(MB, 4 * c + 4)
                psOa = [
                    ps.tile([65, 512], F32, tag="psO", bufs=2, name="psO0"),
                    ps.tile([65, 512], F32, tag="psO", bufs=2, name="psO1"),
                ]
                pts = {}

                def stage_s(jb):
                    psS = ps.tile([P, 1024], F32, tag="psS", bufs=2, name="psS")
                    r = jb - 4 * c
                    pre = P * r if r > 0 else 0
                    for h2 in range(2):
                        nc.tensor.matmul(
                            psS[:, h2 * 512 + pre:(h2 + 1) * 512],
                            kt[hp][h2 * DK:(h2 + 1) * DK, jb * P:(jb + 1) * P],
                            qt[hp][h2 * DK:(h2 + 1) * DK, c * 512 + pre:(c + 1) * 512],
                            start=True,
                            stop=True,
                            tile_position=(h2 * DK, 0),
                        )
                    if r >= 0:
                        # only the 128-wide diagonal triangle needs the additive
                        # mask; fully-masked prefix columns are zeroed after exp
                        for h2 in range(2):
                            nc.vector.tensor_add(
                                psS[:, h2 * 512 + pre:h2 * 512 + pre + P],
                                psS[:, h2 * 512 + pre:h2 * 512 + pre + P],
                                maskt[:],
                            )
                    pt = sb.tile([P, 1024], F16, tag="pt", bufs=4, name="pt")
                    if pre:
                        psS3 = psS[:].rearrange("p (h x) -> p h x", h=2)
                        pt3 = pt[:].rearrange("p (h x) -> p h x", h=2)
                        nc.scalar.activation(
                            pt3[:, :, pre:512], psS3[:, :, pre:512], AF.Exp, scale=0.125
                        )
                        nc.gpsimd.memset(pt3[:, :, 0:pre], 0.0)
                    else:
                        nc.scalar.activation(pt[:], psS[:], AF.Exp, scale=0.125)
                    pts[jb] = pt

                def stage_pv(jb):
                    pt = pts.pop(jb)
                    first, last = (jb == 0), (jb == jb_max - 1)
                    r = jb - 4 * c
                    pre = P * r if (r > 0 and not first) else 0
                    for h2 in range(2):
                        h = 2 * hp + h2
                        nc.tensor.matmul(
                            psOa[h2][0:65, pre:512],
                            v[jb][:, h * 65:(h + 1) * 65],
                            pt[:, h2 * 512 + pre:(h2 + 1) * 512],
                            start=first,
                            stop=last,
                            skip_group_check=True,
                        )
                    pump()

                for jb in range(jb_max):
                    stage_s(jb)
                    if jb >= 2:
                        stage_pv(jb - 2)
                stage_pv(jb_max - 2)
                stage_pv(jb_max - 1)

                cpO = [
                    sb.tile([65, 512], F32, tag="cpo", bufs=4, name=f"cpO{h2}")
                    for h2 in range(2)
                ]
                nc.vector.tensor_copy(cpO[0][0:65, :], psOa[0][0:65, :])
                nc.vector.tensor_copy(cpO[1][0:65, :], psOa[1][0:65, :])
                rbc = [
                    sb.tile([64, 512], F32, tag="rbc", bufs=4, name=f"rbc{h2}")
                    for h2 in range(2)
                ]
                tmp1 = sb.tile([64, 512], F16, tag="tmp1", bufs=2, name="tmp1")
                nr = sb.tile([1, 1024], F32, tag="nr", bufs=4, name="nr")
                nr2 = sb.tile([1, 1024], F32, tag="nr", bufs=4, name="nr2")

                def norm_piece(stage):
                    if stage == 0:
                        # move denominator rows (lane 64) to lane 0
                        nc.sync.dma_start(nr[0:1, 0:512], cpO[0][64:65, :])
                        nc.sync.dma_start(nr[0:1, 512:1024], cpO[1][64:65, :])
                    elif stage == 1:
                        nc.vector.reciprocal_approx_fast(nr2[0:1, :], nr[0:1, :])
                    elif stage == 2:
                        nc.gpsimd.partition_broadcast(rbc[0][0:64, :], nr2[0:1, 0:512])
                        nc.gpsimd.partition_broadcast(rbc[1][0:64, :], nr2[0:1, 512:1024])
                    elif stage == 3:
                        nc.vector.tensor_tensor(
                            ot[hp][0:64, c * 512:(c + 1) * 512],
                            cpO[0][0:64, :],
                            rbc[0][0:64, :],
                            mybir.AluOpType.mult,
                        )
                    elif stage == 4:
                        nc.vector.tensor_tensor(
                            tmp1[0:64, :],
                            cpO[1][0:64, :],
                            rbc[1][0:64, :],
                            mybir.AluOpType.mult,
                        )
                        nc.sync.dma_start(
                            ot[hp][64:128, c * 512:(c + 1) * 512], tmp1[0:64, :]
                        )

                if eager_norm:
                    for st in range(5):
                        norm_piece(st)
                else:
                    for st in range(5):
                        dve_q.append(lambda st=st: norm_piece(st))

            # ---- prologue: v[0..3] + Q/K chunk 0 for all head-pairs,
            # DMA descriptors dribbled in just before their consumers --------
            for m in range(4):
                if m < 3:
                    dma_xtf_m(m + 1)
                if m == 1:
                    nc.scalar.dma_start(wqf[:], wqf_d.ap())
                if m == 2:
                    nc.scalar.dma_start(wkf[:], wkf_d.ap())
                for th in v_proj_parts(m):
                    th()
            for hp in range(NHP):
                for th in qk_proj_parts(hp, 0, wqf, qt[hp]):
                    th()
                for th in qk_proj_parts(hp, 0, wkf, kt[hp]):
                    th()
                for m in range(4 + 3 * hp, min(MB, 7 + 3 * hp)):
                    dma_xtf_m(m, eng=nc.gpsimd)
            nc.gpsimd.dma_start(wof[:], wof_d.ap())

            # ---- waves 0 and 1: tensor-bound, pump next wave's projections --
            for c in range(2):
                fl = []
                for i, hp in enumerate(range(NHP)):
                    fl.extend(v_proj_parts(4 * (c + 1) + i))
                    fl.extend(qk_proj_parts(hp, c + 1, wqf, qt[hp]))
                    fl.extend(qk_proj_parts(hp, c + 1, wkf, kt[hp]))
                filler[:] = [(None, th) for th in fl]
                wave_state["quota"] = len(fl)
                wave_state["popped"] = 0
                wave_state["done"] = 0
                wave_state["total"] = NHP * (4 * c + 4)
                for hp in range(NHP):
                    attn_chunk(hp, c)
                while filler:
                    filler.pop(0)[1]()

            # ---- merged stretch: chunks 2+3 interleaved to even out the exp
            # (ACT) density; qk(*,3) and v[12..15] are pumped just-in-time with
            # deadlines, W_O chunks 0/1 fill the ACT-bound remainder.
            while dve_q:
                dve_q.pop(0)()  # chunk-0/1 norms: emit before their outproj
            order = [(2, 0), (2, 1), (3, 0), (2, 2), (3, 1), (3, 2), (2, 3), (3, 3)]
            slot_of = {u: i for i, u in enumerate(order)}
            fl = []
            for hp in range(NHP):
                ddl = slot_of[(3, hp)]
                for th in qk_proj_parts(hp, 3, wqf, qt[hp]):
                    fl.append((ddl, th))
                for th in qk_proj_parts(hp, 3, wkf, kt[hp]):
                    fl.append((ddl, th))
                for th in v_proj_parts(12 + hp):
                    fl.append((slot_of[(3, 0)], th))
            for cc in range(2):
                for do in range(D // P):
                    fl.append((None, outproj_unit(do, cc)))
            filler[:] = fl
            wave_state["quota"] = len(fl)
            wave_state["popped"] = 0
            wave_state["done"] = 0
            wave_state["total"] = sum(4 * c + 4 for c, _ in order)
            for i, (c, hp) in enumerate(order):
                # deadline drain: everything this or an earlier slot needs
                filler.sort(key=lambda it: (it[0] is None, it[0] if it[0] is not None else 0))
                while filler and filler[0][0] is not None and filler[0][0] <= i:
                    it = filler.pop(0)
                    it[1]()
                    wave_state["popped"] += 1
                attn_chunk(hp, c, eager_norm=((c, hp) == (3, 3)))
                if (c, hp) == (2, 3):
                    # all chunk-2 norms emitted after this; add its outproj
                    while dve_q:
                        dve_q.pop(0)()
                    for do in range(D // P):
                        filler.append((None, outproj_unit(do, 2)))
                    wave_state["quota"] += D // P
            while filler:
                filler.pop(0)[1]()

            # ---- epilogue: last chunk's W_O projection ----------------------
            while dve_q:
                dve_q.pop(0)()
            for do in range(D // P):
                outproj_unit(do, MC - 1)()

    nc.compile()
    return nc


def make_core_inputs(X, mask, Wq, Wk, Wv, Wo):
    """Full inputs -> list of 8 per-core input maps (batch-major, head-group minor)."""
    B = X.shape[0]
    N = X.shape[1]
    maskt = np.ascontiguousarray(mask[0:P, 0:P].T.astype(np.float32))
    in_maps = []
    for b in range(B):
        XT = X[b].T.astype(np.float16)  # [D, N]
        xtf = np.ascontiguousarray(
            XT.reshape(KB, P, N // P, P).transpose(1, 2, 0, 3).reshape(P, KB * N)
        )
        for g in range(2):
            sl = slice(g * DH, (g + 1) * DH)
            WQ = Wq[:, sl].astype(np.float16)
            WK = Wk[:, sl].astype(np.float16)
            WV = Wv[:, sl].astype(np.float16)
            WO = Wo[sl, :].astype(np.float16)
            in_maps.append(
                {
                    "xtf": xtf,
                    "wqf": np.ascontiguousarray(
                        WQ.reshape(KB, P, NHP, P).transpose(1, 2, 0, 3).reshape(P, NHP * 1024)
                    ),
                    "wkf": np.ascontiguousarray(
                        WK.reshape(KB, P, NHP, P).transpose(1, 2, 0, 3).reshape(P, NHP * 1024)
                    ),
                    "wvf": np.ascontiguousarray(
                        WV.reshape(KB, P, DH).transpose(1, 0, 2).reshape(P, KB * DH)
                    ),
                    "wof": np.ascontiguousarray(
                        WO.reshape(NHP, P, D).transpose(1, 0, 2).reshape(P, NHP * D)
                    ),
                    "maskt": maskt,
                    "ones16": np.ones((P, DK), np.float16),
                }
            )
    return in_maps


def gather_output(results, B=4):
    N = results[0]["outt"].shape[1]
    out = np.empty((B, N, D), np.float32)
    for b in range(B):
        s = results[2 * b]["outt"].astype(np.float32) + results[2 * b + 1][
            "outt"
        ].astype(np.float32)
        out[b] = s.T
    return out


# ---------------------------------------------------------------------------
# Self-contained harness entry: full inputs in, full output out.
# Shards across 8 NeuronCores: core = batch b (4) x head-group g (2 x 8 heads).
# Each core runs a fused flash-style causal MHA for its 8 heads; the host
# sums the two head-group partial outputs per batch (row-parallel W_O).
# ---------------------------------------------------------------------------
from concourse.bass_utils import run_bass_kernel_spmd

_NC_CACHE = {}


def _get_nc():
    if "nc" not in _NC_CACHE:
        _NC_CACHE["nc"] = build(N=2048)
    return _NC_CACHE["nc"]


def kernel(X, mask, Wq, Wk, Wv, Wo):
    X = np.asarray(X, dtype=np.float32)
    mask = np.asarray(mask, dtype=np.float32)
    Wq = np.asarray(Wq, dtype=np.float32)
    Wk = np.asarray(Wk, dtype=np.float32)
    Wv = np.asarray(Wv, dtype=np.float32)
    Wo = np.asarray(Wo, dtype=np.float32)
    in_maps = make_core_inputs(X, mask, Wq, Wk, Wv, Wo)
    nc = _get_nc()
    res = run_bass_kernel_spmd(nc, in_maps, list(range(8)))
    return gather_output(res.results, B=X.shape[0])


# revision 19
# speedup vs baseline: 1.0499x; 1.0499x over previous
"""Fused causal MHA kernel for TRN2, one core = (batch b, head-group g of 8 heads).

Chunk-major wave schedule: all four head-pairs process query-chunk c
("wave c") before moving to chunk c+1, so the W_O projection for chunk c
becomes pumpable filler during wave 3 instead of a starved endgame.  Inputs
arrive via a handful of wide folded DMAs (one descriptor each); the 1/sqrt(dk)
scale rides the exp activation; output is f16.  A few W_O units are held back
past the last attention chunk so its normalization chain is covered by tensor
work instead of a pipeline bubble.

Layouts (per core):
  xtf  [128, 16384]  X[b]^T folded chunk-major:
        xtf[p, cc*4096 + k*512 + c] = X^T[k*128+p, cc*512+c]
  wvf  [128, 4096]   wvf[p, k*512+j]          = Wv[k*128+p, g*512+j]
  wqf/wkf [128,4096] wqf[p, hp*1024+k*128+c]  = Wq[k*128+p, g*512+hp*128+c]
  wof  [128, 4096]   wof[p, dv*1024+o]        = Wo[g*512+dv*128+p, o]
  maskt [128, 128]   mask[0:128, 0:128].T  (0 / -1e9)
  outt [1024, N] f16 partial (X attn Wo_g)^T ; host sums the two
        head-group partials per batch and transposes.

On-chip per head-pair hp: qt/kt [128, N] (h0 d0-63, h1 d0-63 on partitions).
v per seq m-block: [128, 8*65]; S^T per (hp, c, jb): psum [128, 1024];
PV col-packed into psO [65, 512] per head with a ones row for the denom.
"""

import numpy as np
import concourse.bass as bass
import concourse.tile as tile
from concourse import bacc, mybir

F32 = mybir.dt.float32
F16 = mybir.dt.float16
AF = mybir.ActivationFunctionType

P = 128
D = 1024
DH = 512  # head-group width: 8 heads * 64
DK = 64
KB = D // P  # 8 k-blocks
NHP = 4  # head-pairs per core


def build(N=2048):
    MB = N // P  # seq 128-blocks (16)
    MC = N // 512  # seq 512-chunks (4)
    nc = bacc.Bacc("TRN2", target_bir_lowering=False, debug=False)

    xtf_d = nc.dram_tensor("xtf", [P, KB * N], F16, kind="ExternalInput")
    wqf_d = nc.dram_tensor("wqf", [P, NHP * 1024], F16, kind="ExternalInput")
    wkf_d = nc.dram_tensor("wkf", [P, NHP * 1024], F16, kind="ExternalInput")
    wvf_d = nc.dram_tensor("wvf", [P, KB * DH], F16, kind="ExternalInput")
    wof_d = nc.dram_tensor("wof", [P, NHP * D], F16, kind="ExternalInput")
    mask_d = nc.dram_tensor("maskt", [P, P], F32, kind="ExternalInput")
    ones_d = nc.dram_tensor("ones16", [P, DK], F16, kind="ExternalInput")
    out_d = nc.dram_tensor("outt", [D, N], F16, kind="ExternalOutput")

    with tile.TileContext(nc) as tc:
        with (
            tc.tile_pool(name="sb", bufs=1) as sb,
            tc.tile_pool(name="ps", bufs=1, space="PSUM") as ps,
        ):
            # ---- persistent tiles ----
            xtf = sb.tile([P, KB * N], F16, tag="xtf", bufs=1)
            wqf = sb.tile([P, NHP * 1024], F16, tag="wqf", bufs=1)
            wkf = sb.tile([P, NHP * 1024], F16, tag="wkf", bufs=1)
            wvf = sb.tile([P, KB * DH], F16, tag="wvf", bufs=1)
            wof = sb.tile([P, NHP * D], F16, tag="wof", bufs=1)
            maskt = sb.tile([P, P], F32, tag="maskt", bufs=1)
            ones = sb.tile([P, DK], F16, tag="ones", bufs=1)
            v = [sb.tile([P, 8 * 65], F16, tag="v", bufs=MB, name=f"v{m}") for m in range(MB)]
            qt = [sb.tile([P, N], F16, tag="qt", bufs=NHP, name=f"qt{h}") for h in range(NHP)]
            kt = [sb.tile([P, N], F16, tag="kt", bufs=NHP, name=f"kt{h}") for h in range(NHP)]
            ot = [sb.tile([P, N], F16, tag="ot", bufs=NHP, name=f"ot{t}") for t in range(NHP)]

            # ---- input DMAs: few wide descriptors, earliest-needed first ----
            nc.sync.dma_start(wvf[:], wvf_d.ap())
            nc.sync.dma_start(xtf[:, 0:4096], xtf_d.ap()[:, 0:4096])
            nc.sync.dma_start(maskt[:], mask_d.ap())
            nc.sync.dma_start(ones[:], ones_d.ap())
            nc.sync.dma_start(wqf[:], wqf_d.ap())
            nc.sync.dma_start(wkf[:], wkf_d.ap())
            for cc in range(1, MC):
                nc.sync.dma_start(
                    xtf[:, cc * 4096:(cc + 1) * 4096],
                    xtf_d.ap()[:, cc * 4096:(cc + 1) * 4096],
                )
            nc.sync.dma_start(wof[:], wof_d.ap())

            # warm the ACT exp table during the DMA lead-in
            warm = sb.tile([P, DK], F16, tag="warm", bufs=1, name="warm")
            nc.scalar.activation(warm[:], ones[:], AF.Exp)

            # ---- helpers -----------------------------------------------------
            def xts(k, cc, lo, hi):
                """xt[k][:, cc*512+lo : cc*512+hi] in the folded layout."""
                base = cc * 4096 + k * 512
                return xtf[:, base + lo:base + hi]

            def v_proj_parts(m):
                """values projection for seq block m -> v[m]; 2 thunks."""
                cc, off = m // 4, (m % 4) * P
                cell = {}

                def part(k0, k1, fin):
                    if k0 == 0:
                        cell["ps"] = ps.tile([P, 512], F32, tag="proj", bufs=2, name="psV")
                    psV = cell["ps"]
                    for k in range(k0, k1):
                        nc.tensor.matmul(
                            psV[:],
                            xts(k, cc, off, off + P),
                            wvf[:, k * 512:(k + 1) * 512],
                            start=(k == 0),
                            stop=(k == KB - 1),
                        )
                    if fin:
                        v3 = v[m][:].rearrange("p (h x) -> p h x", x=65)
                        nc.vector.tensor_copy(
                            v3[:, :, 0:64], psV[:].rearrange("p (h x) -> p h x", x=64)
                        )
                        nc.gpsimd.tensor_copy(v3[:, :, 64:65], ones[:, 0:8, None])

                return [lambda: part(0, 4, False), lambda: part(4, KB, True)]

            def qk_proj_parts(hp, c, wf, dst):
                """one 512-chunk of the Q or K projection for head-pair hp; 2 thunks."""
                cell = {}

                def part(k0, k1, fin):
                    if k0 == 0:
                        cell["ps"] = ps.tile([P, 512], F32, tag="proj", bufs=2, name="psQ")
                    psQ = cell["ps"]
                    for k in range(k0, k1):
                        nc.tensor.matmul(
                            psQ[:],
                            wf[:, hp * 1024 + k * P:hp * 1024 + (k + 1) * P],
                            xts(k, c, 0, 512),
                            start=(k == 0),
                            stop=(k == KB - 1),
                        )
                    if fin:
                        nc.vector.tensor_copy(dst[:, c * 512:(c + 1) * 512], psQ[:])

                return [lambda: part(0, 4, False), lambda: part(4, KB, True)]

            def outproj_unit(do, c):
                """one 128-row out-dim block of chunk c of the W_O projection."""

                def run():
                    psF = ps.tile([P, 512], F32, tag="proj", bufs=2, name="psF")
                    for dv in range(NHP):
                        nc.tensor.matmul(
                            psF[:],
                            wof[:, dv * D + do * P:dv * D + (do + 1) * P],
                            ot[dv][:, c * 512:(c + 1) * 512],
                            start=(dv == 0),
                            stop=(dv == NHP - 1),
                        )
                    o_sb = sb.tile([P, 512], F16, tag="osb", bufs=4, name="o_sb")
                    nc.vector.tensor_copy(o_sb[:], psF[:])
                    nc.sync.dma_start(
                        out_d.ap()[do * P:(do + 1) * P, c * 512:(c + 1) * 512],
                        o_sb[:],
                    )

                return run

            # ---- pump state --------------------------------------------------
            filler = []  # tensor-work thunks allotted to the current wave
            dve_q = []  # norm-chain stages, popped one per attention unit
            wave_state = {"done": 0, "total": 1, "quota": 0, "popped": 0}

            def pump():
                if dve_q:
                    dve_q.pop(0)()
                wave_state["done"] += 1
                target = wave_state["quota"] * wave_state["done"] // wave_state["total"]
                while wave_state["popped"] < target and filler:
                    filler.pop(0)()
                    wave_state["popped"] += 1

            # ---- attention chunk (hp, c) ------------------------------------
            def attn_chunk(hp, c, eager_norm=False):
                jb_max = min(MB, 4 * c + 4)
                psOa = [
                    ps.tile([65, 512], F32, tag="psO", bufs=2, name="psO0"),
                    ps.tile([65, 512], F32, tag="psO", bufs=2, name="psO1"),
                ]
                pts = {}

                def stage_s(jb):
                    psS = ps.tile([P, 1024], F32, tag="psS", bufs=2, name="psS")
                    r = jb - 4 * c
                    pre = P * r if r > 0 else 0
                    for h2 in range(2):
                        nc.tensor.matmul(
                            psS[:, h2 * 512 + pre:(h2 + 1) * 512],
                            kt[hp][h2 * DK:(h2 + 1) * DK, jb * P:(jb + 1) * P],
                            qt[hp][h2 * DK:(h2 + 1) * DK, c * 512 + pre:(c + 1) * 512],
                            start=True,
                            stop=True,
                            tile_position=(h2 * DK, 0),
                        )
                    if r >= 0:
                        # only the 128-wide diagonal triangle needs the additive
                        # mask; fully-masked prefix columns are zeroed after exp
                        for h2 in range(2):
                            nc.vector.tensor_add(
                                psS[:, h2 * 512 + pre:h2 * 512 + pre + P],
                                psS[:, h2 * 512 + pre:h2 * 512 + pre + P],
                                maskt[:],
                            )
                    pt = sb.tile([P, 1024], F16, tag="pt", bufs=4, name="pt")
                    if pre:
                        psS3 = psS[:].rearrange("p (h x) -> p h x", h=2)
                        pt3 = pt[:].rearrange("p (h x) -> p h x", h=2)
                        nc.scalar.activation(
                            pt3[:, :, pre:512], psS3[:, :, pre:512], AF.Exp, scale=0.125
                        )
                        nc.gpsimd.memset(pt3[:, :, 0:pre], 0.0)
                    else:
                        nc.scalar.activation(pt[:], psS[:], AF.Exp, scale=0.125)
                    pts[jb] = pt

                def stage_pv(jb):
                    pt = pts.pop(jb)
                    first, last = (jb == 0), (jb == jb_max - 1)
                    r = jb - 4 * c
                    pre = P * r if (r > 0 and not first) else 0
                    for h2 in range(2):
                        h = 2 * hp + h2
                        nc.tensor.matmul(
                            psOa[h2][0:65, pre:512],
                            v[jb][:, h * 65:(h + 1) * 65],
                            pt[:, h2 * 512 + pre:(h2 + 1) * 512],
                            start=first,
                            stop=last,
                            skip_group_check=True,
                        )
                    pump()

                for jb in range(jb_max):
                    stage_s(jb)
                    if jb >= 2:
                        stage_pv(jb - 2)
                stage_pv(jb_max - 2)
                stage_pv(jb_max - 1)

                cpO = [
                    sb.tile([65, 512], F32, tag="cpo", bufs=4, name=f"cpO{h2}")
                    for h2 in range(2)
                ]
                nc.vector.tensor_copy(cpO[0][0:65, :], psOa[0][0:65, :])
                nc.vector.tensor_copy(cpO[1][0:65, :], psOa[1][0:65, :])
                rbc = [
                    sb.tile([64, 512], F32, tag="rbc", bufs=4, name=f"rbc{h2}")
                    for h2 in range(2)
                ]
                tmp1 = sb.tile([64, 512], F16, tag="tmp1", bufs=2, name="tmp1")
                nr = sb.tile([1, 1024], F32, tag="nr", bufs=4, name="nr")
                nr2 = sb.tile([1, 1024], F32, tag="nr", bufs=4, name="nr2")

                def norm_piece(stage):
                    if stage == 0:
                        # move denominator rows (lane 64) to lane 0
                        nc.sync.dma_start(nr[0:1, 0:512], cpO[0][64:65, :])
                        nc.sync.dma_start(nr[0:1, 512:1024], cpO[1][64:65, :])
                    elif stage == 1:
                        nc.vector.reciprocal_approx_fast(nr2[0:1, :], nr[0:1, :])
                    elif stage == 2:
                        nc.gpsimd.partition_broadcast(rbc[0][0:64, :], nr2[0:1, 0:512])
                        nc.gpsimd.partition_broadcast(rbc[1][0:64, :], nr2[0:1, 512:1024])
                    elif stage == 3:
                        nc.vector.tensor_tensor(
                            ot[hp][0:64, c * 512:(c + 1) * 512],
                            cpO[0][0:64, :],
                            rbc[0][0:64, :],
                            mybir.AluOpType.mult,
                        )
                    elif stage == 4:
                        nc.vector.tensor_tensor(
                            tmp1[0:64, :],
                            cpO[1][0:64, :],
                            rbc[1][0:64, :],
                            mybir.AluOpType.mult,
                        )
                        nc.sync.dma_start(
                            ot[hp][64:128, c * 512:(c + 1) * 512], tmp1[0:64, :]
                        )

                if eager_norm:
                    for st in range(5):
                        norm_piece(st)
                else:
                    for st in range(5):
                        dve_q.append(lambda st=st: norm_piece(st))

            # ---- prologue: v[0..3] + Q/K chunk 0 for all head-pairs ----------
            for m in range(4):
                for th in v_proj_parts(m):
                    th()
            for hp in range(NHP):
                for th in qk_proj_parts(hp, 0, wqf, qt[hp]):
                    th()
                for th in qk_proj_parts(hp, 0, wkf, kt[hp]):
                    th()

            # ---- waves -------------------------------------------------------
            for c in range(MC):
                fl = []
                if c < MC - 1:
                    for i, hp in enumerate(range(NHP)):
                        fl.extend(v_proj_parts(4 * (c + 1) + i))
                        fl.extend(qk_proj_parts(hp, c + 1, wqf, qt[hp]))
                        fl.extend(qk_proj_parts(hp, c + 1, wkf, kt[hp]))
                    reserve = 0
                else:
                    # wave 3: all W_O projection work for chunks 0..2.
                    # emission-drain the norm queue first so ot writes precede
                    # the outproj reads in program order.  Hold a few units
                    # back so the last norm chain overlaps tensor work.
                    while dve_q:
                        dve_q.pop(0)()
                    for cc in range(MC - 1):
                        for do in range(D // P):
                            fl.append(outproj_unit(do, cc))
                    reserve = 6
                filler[:] = fl
                wave_state["quota"] = len(fl) - reserve
                wave_state["popped"] = 0
                wave_state["done"] = 0
                wave_state["total"] = NHP * min(MB, 4 * c + 4)

                for hp in range(NHP):
                    attn_chunk(hp, c, eager_norm=(c == MC - 1 and hp == NHP - 1))
                while filler:
                    filler.pop(0)()

            # ---- epilogue: last chunk's W_O projection ----------------------
            while dve_q:
                dve_q.pop(0)()
            for do in range(D // P):
                outproj_unit(do, MC - 1)()

    nc.compile()
    return nc


def make_core_inputs(X, mask, Wq, Wk, Wv, Wo):
    """Full inputs -> list of 8 per-core input maps (batch-major, head-group minor)."""
    B = X.shape[0]
    N = X.shape[1]
    maskt = np.ascontiguousarray(mask[0:P, 0:P].T.astype(np.float32))
    in_maps = []
    for b in range(B):
        XT = X[b].T.astype(np.float16)  # [D, N]
        xtf = np.ascontiguousarray(
            XT.reshape(KB, P, N // 512, 512).transpose(1, 2, 0, 3).reshape(P, KB * N)
        )
        for g in range(2):
            sl = slice(g * DH, (g + 1) * DH)
            WQ = Wq[:, sl].astype(np.float16)
            WK = Wk[:, sl].astype(np.float16)
            WV = Wv[:, sl].astype(np.float16)
            WO = Wo[sl, :].astype(np.float16)
            in_maps.append(
                {
                    "xtf": xtf,
                    "wqf": np.ascontiguousarray(
                        WQ.reshape(KB, P, NHP, P).transpose(1, 2, 0, 3).reshape(P, NHP * 1024)
                    ),
                    "wkf": np.ascontiguousarray(
                        WK.reshape(KB, P, NHP, P).transpose(1, 2, 0, 3).reshape(P, NHP * 1024)
                    ),
                    "wvf": np.ascontiguousarray(
                        WV.reshape(KB, P, DH).transpose(1, 0, 2).reshape(P, KB * DH)
                    ),
                    "wof": np.ascontiguousarray(
                        WO.reshape(NHP, P, D).transpose(1, 0, 2).reshape(P, NHP * D)
                    ),
                    "maskt": maskt,
                    "ones16": np.ones((P, DK), np.float16),
                }
            )
    return in_maps


def gather_output(results, B=4):
    N = results[0]["outt"].shape[1]
    out = np.empty((B, N, D), np.float32)
    for b in range(B):
        s = results[2 * b]["outt"].astype(np.float32) + results[2 * b + 1][
            "outt"
        ].astype(np.float32)
        out[b] = s.T
    return out


# ---------------------------------------------------------------------------
# Self-contained harness entry: full inputs in, full output out.
# Shards across 8 NeuronCores: core = batch b (4) x head-group g (2 x 8 heads).
# Each core runs a fused flash-style causal MHA for its 8 heads; the host
# sums the two head-group partial outputs per batch (row-parallel W_O).
# ---------------------------------------------------------------------------
from concourse.bass_utils import run_bass_kernel_spmd

_NC_CACHE = {}


def _get_nc():
    if "nc" not in _NC_CACHE:
        _NC_CACHE["nc"] = build(N=2048)
    return _NC_CACHE["nc"]


def kernel(X, mask, Wq, Wk, Wv, Wo):
    X = np.asarray(X, dtype=np.float32)
    mask = np.asarray(mask, dtype=np.float32)
    Wq = np.asarray(Wq, dtype=np.float32)
    Wk = np.asarray(Wk, dtype=np.float32)
    Wv = np.asarray(Wv, dtype=np.float32)
    Wo = np.asarray(Wo, dtype=np.float32)
    in_maps = make_core_inputs(X, mask, Wq, Wk, Wv, Wo)
    nc = _get_nc()
    res = run_bass_kernel_spmd(nc, in_maps, list(range(8)))
    return gather_output(res.results, B=X.shape[0])
